# revision 21
# baseline (speedup 1.0000x reference)
"""CRF negative log-likelihood on 8 Trainium2 NeuronCores.

Strategy
--------
The dominant cost is the forward algorithm (log-partition): a length-T
recurrence of "log-matmuls"  alpha_t = em_t + LSE_i(alpha_{t-1} + trans).
In exp-domain this is  u_t = ehat_t * (expT^T @ u_{t-1}), i.e. a 128x128
matmul + elementwise multiply per step, where ehat = exp(em - CSHIFT) is
precomputed on the host, stored fp8e4m3 in DRAM (halves HBM traffic)
and cast to bf16 in-flight by GPSIMD-initiated (SWDGE) DMAs.

transitions are in [-0.1, 0.1], so exp(trans) is a strong Hilbert-metric
contraction (factor ~tanh(0.1) ~ 0.1/step): the recurrence forgets its
initial condition in a couple of steps. We split T into C=64 chunks per
core and run all chunks in lockstep as columns of ONE state block
[128 states x C*32 cols], each chunk warm-started from a ones vector
(no warmup steps at all; the per-chunk log-gain ratio cancels the
warm-start scale, and the entry-functional mismatch after one step is
< 0.2 in log - far inside the 2e-2 NLL tolerance; measured ~1e-4 rel).

The state block is split into independent column units, each with its
own PSUM/v/y tiles so their mm -> multiply -> mm pipelines share
nothing but engines. The elementwise multiply is the scarce resource
(only DVE and ACT can read PSUM): the direct unit muls on DVE straight
from PSUM (1x mode); evac units have ACT copy PSUM->SBUF bf16 and DVE
mul bf16 at 2x. Evac units are emitted with a one-step skew so their
ACT copies interleave instead of serializing inside one step's critical
path. Chunk 0 is exact: its v-init is zero and a rank-1 ones matmul
accumulates 1.0 into its PSUM columns at s=1, so v(1) =
ehat(t=0)*exp(start) (start folded in on host).

Per-chunk boundary sums (1^T v and exp(end)^T v at s=TC) are computed
with a [K,2] matmul and telescoped into log_Z on the host in f64. The
gold-path score (pure gathers, ~0.006% of FLOPs) and the final mean are
computed on the host.

Sharding: data-parallel over batch B: core i owns b in [32*i, 32*i+32).
"""

import numpy as np
from contextlib import ExitStack

import ml_dtypes

import concourse.bass as bass
import concourse.tile as tile
from concourse import bacc, mybir
from concourse.bass_utils import run_bass_kernel_spmd

# Problem shape (hardcoded per harness contract).
B, T, K = 256, 1024, 128
N_CORES = 8
BC = B // N_CORES          # 32 batch rows per core
C = 64                     # time chunks per core
TC = T // C                # 16 steps per chunk
NV = TC                    # matmul virtual-steps (no warmup)
COLS = C * BC              # 2048 state columns per core
# Per-step rescale: log(128)+0.5 keeps the state O(1); the -3 biases
# ehat up by e^3 so fp8e4m3 subnormals (<2^-9) are never hit. State
# grows e^(3*16)=e^48 over a chunk - comfortably inside f32/bf16 range.
CSHIFT = float(np.log(128.0) + 0.5) - 3.0

F32 = mybir.dt.float32
BF16 = mybir.dt.bfloat16
FP8 = mybir.dt.float8e4
NP_BF16 = ml_dtypes.bfloat16
NP_FP8 = ml_dtypes.float8_e4m3

# Column units: (kind, width, emission skew in steps). Distinct skews
# on the evac units round-robin their ACT copies so ACT never stalls
# inside one step's critical path.
UNITS = (("direct", 512, 0), ("evac", 512, 0), ("evac", 512, 1),
         ("evac", 512, 1))
BANK_N = 512               # PSUM bank capacity in f32 cols
# Step 1 ships bf16 via the fast HWDGE path (quick pipeline start);
# steps 2..NV ship fp8 via SWDGE cast DMAs in these step-blocks.
DMA_BLOCKS = (3, 6, 6)
EH_BUFS = 2

_NC_CACHE = None


def _build_program(repeat=1):
    """Build the per-core SPMD Bass program (identical on all cores).

    repeat > 1 wraps the whole computation in an on-device loop - used
    only by the test harness for differential HW timing.
    """
    nc = bacc.Bacc("TRN2", target_bir_lowering=False, debug=False,
                   num_devices=N_CORES)

    ehat0 = nc.dram_tensor("ehat0", [K, COLS], BF16,
                           kind="ExternalInput").ap()
    ehat = nc.dram_tensor("ehat", [K, (NV - 1) * COLS], FP8,
                          kind="ExternalInput").ap()
    # wts: [expT (K cols) | ones | exp(end)]
    wts = nc.dram_tensor("wts", [K, K + 2], BF16, kind="ExternalInput").ap()
    sums = nc.dram_tensor("sums", [2, COLS], F32, kind="ExternalOutput").ap()

    assert sum(DMA_BLOCKS) == NV - 1
    n_units = len(UNITS)
    u_starts = np.cumsum([0] + [w for _, w, _ in UNITS]).tolist()
    assert u_starts[-1] == COLS
    max_skew = max(sk for _, _, sk in UNITS)

    with tile.TileContext(nc) as tc, ExitStack() as ctx:
        const_pool = ctx.enter_context(tc.tile_pool(name="const", bufs=1))
        eh_pool = ctx.enter_context(tc.tile_pool(name="eh", bufs=EH_BUFS))
        v_pool = ctx.enter_context(tc.tile_pool(name="v", bufs=2))
        y_pool = ctx.enter_context(tc.tile_pool(name="y", bufs=2))
        ps_pool = ctx.enter_context(
            tc.tile_pool(name="ps", bufs=1, space="PSUM"))
        bs_pool = ctx.enter_context(
            tc.tile_pool(name="bs", bufs=2, space="PSUM"))

        wts_sb = const_pool.tile([K, K + 2], BF16)
        ones1 = const_pool.tile([1, K], BF16)     # rank-1 lhsT (ones)
        nc.vector.memset(ones1[:], 1.0)
        onesBC = const_pool.tile([1, BC], BF16)   # rank-1 rhs (ones)
        nc.vector.memset(onesBC[:], 1.0)
        v0 = const_pool.tile([K, COLS], BF16)     # warm-start state
        nc.vector.memset(v0[:], 1.0)
        nc.vector.memset(v0[:, 0:BC], 0.0)  # chunk 0: exact init via rank-1
        out_sb = const_pool.tile([2, COLS], F32)

        loop_cm = tc.For_i(0, repeat, 1) if repeat > 1 else None
        if loop_cm is not None:
            ctx.enter_context(loop_cm)

        ps_tiles = [ps_pool.tile([K, w], F32, name=f"ps{i}", tag=f"ps{i}")
                    for i, (_, w, _) in enumerate(UNITS)]

        # Step 1: bf16 via HWDGE (fast start); rest: fp8 in DRAM, cast
        # to bf16 by SWDGE (gpsimd) DMAs.
        eh0_t = eh_pool.tile([K, COLS], BF16, tag="eh0", bufs=1)
        nc.sync.dma_start(eh0_t[:], ehat0[:])
        nc.sync.dma_start(wts_sb[:], wts[:])
        eh_tiles = []
        s_lo = 0
        max_blk = max(DMA_BLOCKS)
        for nsteps in DMA_BLOCKS:
            eh_t = eh_pool.tile([K, max_blk * COLS], BF16, tag="eh")
            nc.gpsimd.dma_start(eh_t[:, 0:nsteps * COLS],
                                ehat[:, s_lo * COLS:(s_lo + nsteps) * COLS])
            eh_tiles.append((eh_t, s_lo + 1))
            s_lo += nsteps
        expT = wts_sb[:, 0:K]
        onesend = wts_sb[:, K:K + 2]

        def eh_slice(s, c0, c1):
            if s == 1:
                return eh0_t[:, c0:c1]
            for (eh_t, base), nsteps in zip(eh_tiles, DMA_BLOCKS):
                if base < s <= base + nsteps:
                    off = s - 1 - base
                    return eh_t[:, off * COLS + c0:off * COLS + c1]
            raise AssertionError(s)

        v_cur = [v0[:, u_starts[i]:u_starts[i + 1]] for i in range(n_units)]

        def emit_unit(i, s):
            kind, w, _ = UNITS[i]
            c0 = u_starts[i]
            ps = ps_tiles[i]
            first = (s == 1)
            e_t = eh_slice(s, c0, c0 + w)
            vp = v_cur[i]
            m0 = 0
            while m0 < w:
                m1 = min(w, m0 + BANK_N)
                nc.tensor.matmul(ps[:, m0:m1], expT, vp[:, m0:m1],
                                 start=True,
                                 stop=not (first and i == 0 and m0 == 0),
                                 skip_group_check=first and i == 0)
                if first and i == 0 and m0 == 0:
                    # chunk 0 exact init: ps[:, 0:BC] = 0 + outer(1,1)
                    nc.tensor.matmul(ps[:, 0:BC], ones1[:], onesBC[:],
                                     start=False, stop=True,
                                     skip_group_check=True)
                m0 = m1
            vn = v_pool.tile([K, w], BF16, name=f"v{i}", tag=f"v{i}")
            if kind == "direct":
                nc.vector.tensor_mul(vn[:], ps[:], e_t)
            else:
                y = y_pool.tile([K, w], BF16, name=f"y{i}", tag=f"y{i}")
                nc.scalar.activation(y[:], ps[:],
                                     mybir.ActivationFunctionType.Copy)
                nc.vector.tensor_mul(vn[:], y[:], e_t)
            v_cur[i] = vn

        def emit_bsum(i):
            # final boundary sums for unit i: [1^T v ; exp(end)^T v]
            c0 = u_starts[i]
            w = UNITS[i][1]
            bs = bs_pool.tile([2, w], F32, name="bs", tag="bs")
            nc.tensor.matmul(bs[:], onesend[:], v_cur[i][:], start=True,
                             stop=True)
            if i % 2 == 0:
                nc.scalar.activation(out_sb[0:2, c0:c0 + w], bs[:],
                                     mybir.ActivationFunctionType.Copy)
            else:
                nc.vector.tensor_copy(out_sb[0:2, c0:c0 + w], bs[:])

        for it in range(1, NV + 1 + max_skew):
            for i, (kind, w, skew) in enumerate(UNITS):
                s = it - skew
                if 1 <= s <= NV:
                    emit_unit(i, s)
                    if s == NV:
                        emit_bsum(i)

        nc.sync.dma_start(sums[:], out_sb[:])

    nc.compile()
    return nc


def _host_prep(emissions, start_transitions):
    """Per-core ehat layout: ehat[k, (s-1)*COLS + c*BC + b]
    = exp(em[core*BC + b, c*TC + s - 1, k] - CSHIFT), with start folded
    into t=0. Step 1 ships bf16 (ehat0), steps 2..NV fp8e4m3 (ehat)."""
    em = np.asarray(emissions, dtype=np.float32)
    em = em - CSHIFT
    em[:, 0, :] += start_transitions[None, :].astype(np.float32)
    eh = np.exp(em, dtype=np.float32)                  # [B, T, K] f32
    in_maps = []
    for core in range(N_CORES):
        ehc = eh[core * BC:(core + 1) * BC]            # [BC, T, K]
        ehc = ehc.reshape(BC, C, TC, K)
        # target [K, TC(s), C, BC]
        emx = np.ascontiguousarray(ehc.transpose(3, 2, 1, 0))
        emx = emx.reshape(K, NV, COLS)
        in_maps.append({
            "ehat0": np.ascontiguousarray(emx[:, 0, :]).astype(NP_BF16),
            "ehat": np.ascontiguousarray(
                emx[:, 1:, :].reshape(K, (NV - 1) * COLS)).astype(NP_FP8),
        })
    return in_maps


def _make_in_maps(inputs):
    """Build per-core device input maps from the full input dict."""
    in_maps = _host_prep(
        np.ascontiguousarray(np.asarray(inputs["emissions"],
                                        dtype=np.float32)),
        np.asarray(inputs["start_transitions"], dtype=np.float32))
    wts_in = _make_wts(
        np.asarray(inputs["transitions"], dtype=np.float32),
        np.asarray(inputs["end_transitions"], dtype=np.float32))
    for m in in_maps:
        m["wts"] = wts_in
    return in_maps


def _make_wts(transitions, end_transitions):
    w = np.empty((K, K + 2), dtype=NP_BF16)
    w[:, 0:K] = np.exp(transitions.astype(np.float32)).astype(NP_BF16)
    w[:, K] = np.ones(K, dtype=NP_BF16)
    w[:, K + 1] = np.exp(end_transitions.astype(np.float32)).astype(NP_BF16)
    return w


def _assemble_logz(results):
    """Telescoped per-chunk log-gains; entry sums are exactly K."""
    logz = np.zeros(B)
    logK = np.log(float(K))
    for core in range(N_CORES):
        r = np.asarray(results[core]["sums"], dtype=np.float64)
        end0 = r[0].reshape(C, BC)
        end1 = r[1].reshape(C, BC)
        acc = np.log(end0[0]).copy()                  # chunk 0: exact scale
        for c in range(1, C - 1):
            acc += np.log(end0[c]) - logK
        acc += np.log(end1[C - 1]) - logK             # last: exp(end)^T
        logz[core * BC:(core + 1) * BC] = acc + T * CSHIFT
    return logz


def _gold_score(em, tags, mask, trans, start, end):
    em = em.astype(np.float64)
    mask = mask.astype(np.float64)
    tg = tags.astype(np.int64)
    score = start.astype(np.float64)[tg[:, 0]]
    emit = np.take_along_axis(em, tg[:, :, None], axis=2)[:, :, 0]
    score = score + (emit * mask).sum(axis=1)
    score = score + (trans.astype(np.float64)[tg[:, :-1], tg[:, 1:]]
                     * mask[:, 1:]).sum(axis=1)
    seq_ends = mask.astype(np.int64).sum(axis=1) - 1
    last = tg[np.arange(tg.shape[0]), seq_ends]
    score = score + end.astype(np.float64)[last]
    return score


def _host_logz_fallback(em, trans, start, end):
    """Exact f64 forward algorithm (only used if mask is not all-ones)."""
    em = em.astype(np.float64)
    la = start.astype(np.float64) + em[:, 0, :]
    tr = trans.astype(np.float64)
    for t in range(1, em.shape[1]):
        sc = tr[None] + la[:, :, None] + em[:, t, None, :]
        m = sc.max(axis=1, keepdims=True)
        la = np.squeeze(m, 1) + np.log(np.exp(sc - m).sum(axis=1))
    x = la + end[None].astype(np.float64)
    m = x.max(axis=1, keepdims=True)
    return np.squeeze(m, 1) + np.log(np.exp(x - m).sum(axis=1))


def kernel(emissions, tags, mask, transitions, start_transitions,
           end_transitions):
    global _NC_CACHE
    emissions = np.ascontiguousarray(np.asarray(emissions, dtype=np.float32))
    tags = np.asarray(tags)
    mask = np.asarray(mask)
    transitions = np.asarray(transitions, dtype=np.float32)
    start_transitions = np.asarray(start_transitions, dtype=np.float32)
    end_transitions = np.asarray(end_transitions, dtype=np.float32)

    score = _gold_score(emissions, tags, mask, transitions,
                        start_transitions, end_transitions)

    if not np.all(mask == 1):
        logz = _host_logz_fallback(emissions, transitions,
                                   start_transitions, end_transitions)
        return np.float32(-(score - logz).mean())

    if _NC_CACHE is None:
        _NC_CACHE = _build_program()
    nc = _NC_CACHE

    in_maps = _make_in_maps({
        "emissions": emissions,
        "start_transitions": start_transitions,
        "transitions": transitions,
        "end_transitions": end_transitions,
    })

    results = run_bass_kernel_spmd(nc, in_maps, list(range(N_CORES))).results
    logz = _assemble_logz(results)
    return np.float32(-(score - logz).mean())


# revision 36
# speedup vs baseline: 2.7773x; 2.7773x over previous
"""CRF negative log-likelihood on 8 Trainium2 NeuronCores.

Strategy
--------
The dominant cost is the forward algorithm (log-partition): a length-T
recurrence of "log-matmuls"  alpha_t = em_t + LSE_i(alpha_{t-1} + trans).
In exp-domain this is  u_t = ehat_t * (expT^T @ u_{t-1}), i.e. a 128x128
matmul + elementwise multiply per step, where ehat = exp(em - CSHIFT) is
precomputed on the host, stored fp8e4m3 in DRAM (halves HBM traffic)
and cast to bf16 in-flight by GPSIMD-initiated (SWDGE) DMAs.

transitions are in [-0.1, 0.1], so exp(trans) is a strong Hilbert-metric
contraction (factor ~tanh(0.1) ~ 0.1/step): the recurrence forgets its
initial condition in a couple of steps. We split T into C=64 chunks per
core and run all chunks in lockstep as columns of ONE state block
[128 states x C*32 cols], each chunk warm-started from a ones vector
(no warmup steps at all; the per-chunk log-gain ratio cancels the
warm-start scale, and the entry-functional mismatch after one step is
< 0.2 in log - far inside the 2e-2 NLL tolerance; measured ~1e-4 rel).

The state block is split into independent column units, each with its
own PSUM/v/y tiles so their mm -> multiply -> mm pipelines share
nothing but engines. The elementwise multiply is the scarce resource
(only DVE and ACT can read PSUM): the direct unit muls on DVE straight
from PSUM (1x mode); evac units have ACT copy PSUM->SBUF bf16 and DVE
mul bf16 at 2x. Evac units are emitted with a one-step skew so their
ACT copies interleave instead of serializing inside one step's critical
path. Chunk 0 is exact: its v-init is zero and a rank-1 ones matmul
accumulates 1.0 into its PSUM columns at s=1, so v(1) =
ehat(t=0)*exp(start) (start folded in on host).

Per-chunk boundary sums (1^T v and exp(end)^T v at s=TC) are computed
with a [K,2] matmul and telescoped into log_Z on the host in f64. The
gold-path score (pure gathers, ~0.006% of FLOPs) and the final mean are
computed on the host.

Sharding: data-parallel over batch B: core i owns b in [32*i, 32*i+32).
"""

import numpy as np
from contextlib import ExitStack

import ml_dtypes

import concourse.bass as bass
import concourse.tile as tile
from concourse import bacc, mybir
from concourse.bass_utils import run_bass_kernel_spmd

# Problem shape (hardcoded per harness contract).
B, T, K = 256, 1024, 128
N_CORES = 8
BC = B // N_CORES          # 32 batch rows per core
C = 64                     # time chunks per core
TC = T // C                # 16 steps per chunk
NV = TC                    # matmul virtual-steps (no warmup)
COLS = C * BC              # 2048 state columns per core
# Per-step rescale: log(128)+0.5 keeps the state O(1); the -3 biases
# ehat up by e^3 so fp8e4m3 subnormals (<2^-9) are never hit. State
# grows e^(3*16)=e^48 over a chunk - comfortably inside f32/bf16 range.
CSHIFT = float(np.log(128.0) + 0.5) - 3.0

F32 = mybir.dt.float32
BF16 = mybir.dt.bfloat16
FP8 = mybir.dt.float8e4
NP_BF16 = ml_dtypes.bfloat16
NP_FP8 = ml_dtypes.float8_e4m3

# Column units: (kind, width, emission skew in steps). Distinct skews
# on the evac units round-robin their ACT copies so ACT never stalls
# inside one step's critical path.
UNITS = (("direct", 512, 0), ("evac", 512, 0), ("evac", 512, 1),
         ("evac", 512, 1))
BANK_N = 512               # PSUM bank capacity in f32 cols
# Step 1 ships separately for a quick pipeline start; steps 2..NV in
# these step-blocks (finer early blocks flow data to the pipeline
# sooner).
DMA_BLOCKS = (1, 2, 3, 4, 5)
DMA_ENGINES = ("sync",) * 5
EH_BUFS = 3
USE_FP8 = False            # fp8 ehat + SWDGE cast DMA vs all-bf16 HWDGE
                           # (measured: SWDGE cast is slower on HW)

_NC_CACHE = None


def _build_program(repeat=1):
    """Build the per-core SPMD Bass program (identical on all cores).

    repeat > 1 wraps the whole computation in an on-device loop - used
    only by the test harness for differential HW timing.
    """
    nc = bacc.Bacc("TRN2", target_bir_lowering=False, debug=False,
                   num_devices=N_CORES)

    ehat0 = nc.dram_tensor("ehat0", [K, COLS], BF16,
                           kind="ExternalInput").ap()
    ehat = nc.dram_tensor("ehat", [K, (NV - 1) * COLS],
                          FP8 if USE_FP8 else BF16,
                          kind="ExternalInput").ap()
    # wts: [expT (K cols) | ones | exp(end)]
    wts = nc.dram_tensor("wts", [K, K + 2], BF16, kind="ExternalInput").ap()
    sums = nc.dram_tensor("sums", [2, COLS], F32, kind="ExternalOutput").ap()

    assert sum(DMA_BLOCKS) == NV - 1
    n_units = len(UNITS)
    u_starts = np.cumsum([0] + [w for _, w, _ in UNITS]).tolist()
    assert u_starts[-1] == COLS
    max_skew = max(sk for _, _, sk in UNITS)

    with tile.TileContext(nc) as tc, ExitStack() as ctx:
        const_pool = ctx.enter_context(tc.tile_pool(name="const", bufs=1))
        eh_pool = ctx.enter_context(tc.tile_pool(name="eh", bufs=EH_BUFS))
        v_pool = ctx.enter_context(tc.tile_pool(name="v", bufs=2))
        y_pool = ctx.enter_context(tc.tile_pool(name="y", bufs=2))
        ps_pool = ctx.enter_context(
            tc.tile_pool(name="ps", bufs=1, space="PSUM"))
        bs_pool = ctx.enter_context(
            tc.tile_pool(name="bs", bufs=2, space="PSUM"))

        wts_sb = const_pool.tile([K, K + 2], BF16)
        ones1 = const_pool.tile([1, K], BF16)     # rank-1 lhsT (ones)
        nc.vector.memset(ones1[:], 1.0)
        onesBC = const_pool.tile([1, BC], BF16)   # rank-1 rhs (ones)
        nc.vector.memset(onesBC[:], 1.0)
        v0 = const_pool.tile([K, COLS], BF16)     # warm-start state
        nc.vector.memset(v0[:], 1.0)
        nc.vector.memset(v0[:, 0:BC], 0.0)  # chunk 0: exact init via rank-1
        out_sb = const_pool.tile([2, COLS], F32)

        loop_cm = tc.For_i(0, repeat, 1) if repeat > 1 else None
        if loop_cm is not None:
            ctx.enter_context(loop_cm)

        ps_tiles = [ps_pool.tile([K, w], F32, name=f"ps{i}", tag=f"ps{i}")
                    for i, (_, w, _) in enumerate(UNITS)]

        # Weights first (tiny), then step 1, then blocks across rings.
        nc.sync.dma_start(wts_sb[:], wts[:])
        eh0_t = eh_pool.tile([K, COLS], BF16, tag="eh0", bufs=1)
        nc.sync.dma_start(eh0_t[:], ehat0[:])
        eh_tiles = []
        s_lo = 0
        max_blk = max(DMA_BLOCKS)
        for nsteps, eng_name in zip(DMA_BLOCKS, DMA_ENGINES):
            eh_t = eh_pool.tile([K, max_blk * COLS], BF16, tag="eh")
            eng = nc.gpsimd if USE_FP8 else getattr(nc, eng_name)
            eng.dma_start(eh_t[:, 0:nsteps * COLS],
                          ehat[:, s_lo * COLS:(s_lo + nsteps) * COLS])
            eh_tiles.append((eh_t, s_lo + 1))
            s_lo += nsteps
        expT = wts_sb[:, 0:K]
        onesend = wts_sb[:, K:K + 2]

        def eh_slice(s, c0, c1):
            if s == 1:
                return eh0_t[:, c0:c1]
            for (eh_t, base), nsteps in zip(eh_tiles, DMA_BLOCKS):
                if base < s <= base + nsteps:
                    off = s - 1 - base
                    return eh_t[:, off * COLS + c0:off * COLS + c1]
            raise AssertionError(s)

        v_cur = [v0[:, u_starts[i]:u_starts[i + 1]] for i in range(n_units)]

        def emit_unit(i, s):
            kind, w, _ = UNITS[i]
            c0 = u_starts[i]
            ps = ps_tiles[i]
            first = (s == 1)
            e_t = eh_slice(s, c0, c0 + w)
            vp = v_cur[i]
            m0 = 0
            while m0 < w:
                m1 = min(w, m0 + BANK_N)
                nc.tensor.matmul(ps[:, m0:m1], expT, vp[:, m0:m1],
                                 start=True,
                                 stop=not (first and i == 0 and m0 == 0),
                                 skip_group_check=first and i == 0)
                if first and i == 0 and m0 == 0:
                    # chunk 0 exact init: ps[:, 0:BC] = 0 + outer(1,1)
                    nc.tensor.matmul(ps[:, 0:BC], ones1[:], onesBC[:],
                                     start=False, stop=True,
                                     skip_group_check=True)
                m0 = m1
            vn = v_pool.tile([K, w], BF16, name=f"v{i}", tag=f"v{i}")
            if kind == "direct":
                nc.vector.tensor_mul(vn[:], ps[:], e_t)
            else:
                y = y_pool.tile([K, w], BF16, name=f"y{i}", tag=f"y{i}")
                nc.scalar.activation(y[:], ps[:],
                                     mybir.ActivationFunctionType.Copy)
                nc.vector.tensor_mul(vn[:], y[:], e_t)
            v_cur[i] = vn

        for it in range(1, NV + 1 + max_skew):
            for i, (kind, w, skew) in enumerate(UNITS):
                s = it - skew
                if 1 <= s <= NV:
                    emit_unit(i, s)

        # final boundary sums: [1^T v ; exp(end)^T v] per 512-col quarter
        for q in range(COLS // BANK_N):
            c0 = q * BANK_N
            bs = bs_pool.tile([2, BANK_N], F32, name="bs", tag="bs")
            m0 = c0
            while m0 < c0 + BANK_N:
                i = max(j for j in range(n_units) if u_starts[j] <= m0)
                m1 = min(c0 + BANK_N, u_starts[i + 1])
                nc.tensor.matmul(bs[0:2, m0 - c0:m1 - c0], onesend[:],
                                 v_cur[i][:, m0 - u_starts[i]:
                                          m1 - u_starts[i]],
                                 start=True, stop=True)
                m0 = m1
            if q % 2 == 0:
                nc.scalar.activation(out_sb[0:2, c0:c0 + BANK_N], bs[:],
                                     mybir.ActivationFunctionType.Copy)
            else:
                nc.vector.tensor_copy(out_sb[0:2, c0:c0 + BANK_N], bs[:])
            if q == 1:
                nc.sync.dma_start(sums[:, 0:2 * BANK_N],
                                  out_sb[:, 0:2 * BANK_N])
        nc.sync.dma_start(sums[:, 2 * BANK_N:COLS],
                          out_sb[:, 2 * BANK_N:COLS])

    nc.compile()
    return nc


def _host_prep(emissions, start_transitions):
    """Per-core ehat layout: ehat[k, (s-1)*COLS + c*BC + b]
    = exp(em[core*BC + b, c*TC + s - 1, k] - CSHIFT), with start folded
    into t=0. Step 1 ships bf16 (ehat0), steps 2..NV fp8e4m3 (ehat)."""
    em = np.asarray(emissions, dtype=np.float32)
    em = em - CSHIFT
    em[:, 0, :] += start_transitions[None, :].astype(np.float32)
    eh = np.exp(em, dtype=np.float32)                  # [B, T, K] f32
    in_maps = []
    for core in range(N_CORES):
        ehc = eh[core * BC:(core + 1) * BC]            # [BC, T, K]
        ehc = ehc.reshape(BC, C, TC, K)
        # target [K, TC(s), C, BC]
        emx = np.ascontiguousarray(ehc.transpose(3, 2, 1, 0))
        emx = emx.reshape(K, NV, COLS)
        in_maps.append({
            "ehat0": np.ascontiguousarray(emx[:, 0, :]).astype(NP_BF16),
            "ehat": np.ascontiguousarray(
                emx[:, 1:, :].reshape(K, (NV - 1) * COLS)).astype(
                    NP_FP8 if USE_FP8 else NP_BF16),
        })
    return in_maps


def _make_in_maps(inputs):
    """Build per-core device input maps from the full input dict."""
    in_maps = _host_prep(
        np.ascontiguousarray(np.asarray(inputs["emissions"],
                                        dtype=np.float32)),
        np.asarray(inputs["start_transitions"], dtype=np.float32))
    wts_in = _make_wts(
        np.asarray(inputs["transitions"], dtype=np.float32),
        np.asarray(inputs["end_transitions"], dtype=np.float32))
    for m in in_maps:
        m["wts"] = wts_in
    return in_maps


def _make_wts(transitions, end_transitions):
    w = np.empty((K, K + 2), dtype=NP_BF16)
    w[:, 0:K] = np.exp(transitions.astype(np.float32)).astype(NP_BF16)
    w[:, K] = np.ones(K, dtype=NP_BF16)
    w[:, K + 1] = np.exp(end_transitions.astype(np.float32)).astype(NP_BF16)
    return w


def _assemble_logz(results):
    """Telescoped per-chunk log-gains; entry sums are exactly K."""
    logz = np.zeros(B)
    logK = np.log(float(K))
    for core in range(N_CORES):
        r = np.asarray(results[core]["sums"], dtype=np.float64)
        end0 = r[0].reshape(C, BC)
        end1 = r[1].reshape(C, BC)
        acc = np.log(end0[0]).copy()                  # chunk 0: exact scale
        for c in range(1, C - 1):
            acc += np.log(end0[c]) - logK
        acc += np.log(end1[C - 1]) - logK             # last: exp(end)^T
        logz[core * BC:(core + 1) * BC] = acc + T * CSHIFT
    return logz


def _gold_score(em, tags, mask, trans, start, end):
    em = em.astype(np.float64)
    mask = mask.astype(np.float64)
    tg = tags.astype(np.int64)
    score = start.astype(np.float64)[tg[:, 0]]
    emit = np.take_along_axis(em, tg[:, :, None], axis=2)[:, :, 0]
    score = score + (emit * mask).sum(axis=1)
    score = score + (trans.astype(np.float64)[tg[:, :-1], tg[:, 1:]]
                     * mask[:, 1:]).sum(axis=1)
    seq_ends = mask.astype(np.int64).sum(axis=1) - 1
    last = tg[np.arange(tg.shape[0]), seq_ends]
    score = score + end.astype(np.float64)[last]
    return score


def _host_logz_fallback(em, trans, start, end):
    """Exact f64 forward algorithm (only used if mask is not all-ones)."""
    em = em.astype(np.float64)
    la = start.astype(np.float64) + em[:, 0, :]
    tr = trans.astype(np.float64)
    for t in range(1, em.shape[1]):
        sc = tr[None] + la[:, :, None] + em[:, t, None, :]
        m = sc.max(axis=1, keepdims=True)
        la = np.squeeze(m, 1) + np.log(np.exp(sc - m).sum(axis=1))
    x = la + end[None].astype(np.float64)
    m = x.max(axis=1, keepdims=True)
    return np.squeeze(m, 1) + np.log(np.exp(x - m).sum(axis=1))


def kernel(emissions, tags, mask, transitions, start_transitions,
           end_transitions):
    global _NC_CACHE
    emissions = np.ascontiguousarray(np.asarray(emissions, dtype=np.float32))
    tags = np.asarray(tags)
    mask = np.asarray(mask)
    transitions = np.asarray(transitions, dtype=np.float32)
    start_transitions = np.asarray(start_transitions, dtype=np.float32)
    end_transitions = np.asarray(end_transitions, dtype=np.float32)

    score = _gold_score(emissions, tags, mask, transitions,
                        start_transitions, end_transitions)

    if not np.all(mask == 1):
        logz = _host_logz_fallback(emissions, transitions,
                                   start_transitions, end_transitions)
        return np.float32(-(score - logz).mean())

    if _NC_CACHE is None:
        _NC_CACHE = _build_program()
    nc = _NC_CACHE

    in_maps = _make_in_maps({
        "emissions": emissions,
        "start_transitions": start_transitions,
        "transitions": transitions,
        "end_transitions": end_transitions,
    })

    results = run_bass_kernel_spmd(nc, in_maps, list(range(N_CORES))).results
    logz = _assemble_logz(results)
    return np.float32(-(score - logz).mean())
